# revision 1
# baseline (speedup 1.0000x reference)
# DenseGATv2Conv Trainium2 kernel.
#
# Math (per batch b):
#   xl = x @ W_l + b_l ; xr = x @ W_r + b_r            [N, H*C]
#   alpha[i,j,h] = sum_c att[h,c] * leaky_relu(xl[j,hc] + xr[i,hc], 0.2)
#   S = softmax_j(alpha masked by adj(+self loops))
#   out[i,hc] = sum_j S[i,j,h] * xr[j,hc] + bias
#
# Key identity used on device:
#   leaky_relu(z) = 0.2*z + 0.8*relu(z)
#   alpha[i,j,h] = 0.2*sl[j,h] + 0.2*sr[i,h] + 0.8*sum_c att[h,c]*relu(xl[j,hc]+xr[i,hc])
# where sl = xl @ att_blk, sr = xr @ att_blk are rank-1 in the (i,j) plane.
# In softmax over j the exp(0.2*sr[i,h]) factor cancels; exp(0.2*sl[j,h]) is
# folded multiplicatively into the aggregation operand. So the only O(N^2*HC)
# work is ONE fused elementwise op per destination row pair: relu(xl[j]+xr[i])
# followed by a tensor-engine contraction with a block-diagonal att matrix.
#
# Sharding: 8 cores = (batch b in 0..1) x (4 blocks of 256 destination rows).
# Each core gets full x[b], its 256-row slice of adj (with self-loops set
# host-side), and returns its [256, 64] slice of the output.

import numpy as np

B, N, F, H, C = 2, 1024, 128, 4, 16
HC = H * C
NCORES = 8
NI = 256          # destination rows per core
NPAIR = NI // 2   # 128 pairs of destination rows
NSUP = 8          # supers of 16 pairs (32 dest rows) each
NEG = 0.2

_CACHE = {}
LAST_RESULTS = None


GEN_ACT_MOD = 4


def _build_program(debug=False):
    import concourse.bass as bass
    import concourse.mybir as mybir
    import concourse.tile as tile
    from concourse import bacc

    f32 = mybir.dt.float32
    f32r = mybir.dt.float32r
    f16 = mybir.dt.float16
    Alu = mybir.AluOpType
    Act = mybir.ActivationFunctionType

    nc = bacc.Bacc(
        "TRN2",
        target_bir_lowering=False,
        debug=False,
        enable_asserts=False,
        num_devices=NCORES,
    )

    # ---- DRAM I/O ----
    xb = nc.dram_tensor("xb", [N, F], f32, kind="ExternalInput").ap()
    xis = nc.dram_tensor("xis", [NI, F], f32, kind="ExternalInput").ap()
    adjs = nc.dram_tensor("adjs", [NI, N], f16, kind="ExternalInput").ap()
    wl = nc.dram_tensor("wl", [F, HC], f32, kind="ExternalInput").ap()
    wr = nc.dram_tensor("wr", [F, HC], f32, kind="ExternalInput").ap()
    blp = nc.dram_tensor("blp", [HC, 1], f32, kind="ExternalInput").ap()
    brp = nc.dram_tensor("brp", [HC, 1], f32, kind="ExternalInput").ap()
    att2p = nc.dram_tensor("att2p", [F, 32], f32, kind="ExternalInput").ap()
    attb = nc.dram_tensor("attb", [HC, H], f32, kind="ExternalInput").ap()
    ident = nc.dram_tensor("ident", [128, 128], f32, kind="ExternalInput").ap()
    biasb = nc.dram_tensor("biasb", [128, HC], f32, kind="ExternalInput").ap()
    out = nc.dram_tensor("out", [NI, HC], f32, kind="ExternalOutput").ap()
    dbg_t = {}
    if debug:
        f16_ = mybir.dt.float16
        for nm, shp, dt_ in [("d_xl2T", [128, N], f32), ("d_xrp", [128, NPAIR], f32),
                             ("d_eslT", [H, N], f32), ("d_xrmod", [128, 544], f32),
                             ("d_adjT", [128, 2 * N], f16_), ("d_rp0", [128, N], f16_),
                             ("d_ssb0", [128, N], f16_), ("d_scomp0", [128, N], f16_),
                             ("d_stt0", [128, 4096], f16_)]:
            dbg_t[nm] = nc.dram_tensor(nm, shp, dt_, kind="ExternalOutput").ap()

    with tile.TileContext(nc) as tc:
        _body(tc, nc, mybir, bass, f32, f32r, Alu, Act,
              f16, xb, xis, adjs, wl, wr, blp, brp, att2p, attb, ident, biasb, out,
              dbg_t)

    nc.compile()
    return nc


def _body(tc, nc, mybir, bass, f32, f32r, Alu, Act,
          f16, xb, xis, adjs, wl, wr, blp, brp, att2p, attb, ident, biasb, out,
          dbg_t=None):
    from contextlib import ExitStack
    ctx = ExitStack()
    with ctx:
        consts = ctx.enter_context(tc.tile_pool(name="consts", bufs=1))
        work = ctx.enter_context(tc.tile_pool(name="work", bufs=1))
        rp_pool = ctx.enter_context(tc.tile_pool(name="rp", bufs=8))
        ssb_pool = ctx.enter_context(tc.tile_pool(name="ssb", bufs=3))
        scomp_pool = ctx.enter_context(tc.tile_pool(name="scomp", bufs=2))
        outp = ctx.enter_context(tc.tile_pool(name="outp", bufs=2))
        psg = ctx.enter_context(tc.tile_pool(name="psg", bufs=2, space="PSUM"))
        pst = ctx.enter_context(tc.tile_pool(name="pst", bufs=4, space="PSUM"))

        dma = nc.sync.dma_start

        # ---------- load constants ----------
        wl_t = consts.tile([F, HC], f32, tag="wl")
        wr_t = consts.tile([F, HC], f32, tag="wr")
        blp_t = consts.tile([HC, 1], f32, tag="blp")
        brp_t = consts.tile([HC, 1], f32, tag="brp")
        att2p_t = consts.tile([F, 32], f32, tag="att2p")
        att2p_r = consts.tile([F, 32], f16, tag="att2pr")
        attb_t = consts.tile([HC, H], f32, tag="attb")
        wl_r = consts.tile([F, HC], f32r, tag="wlr")
        wr_r = consts.tile([F, HC], f32r, tag="wrr")
        attb_r = consts.tile([HC, H], f32r, tag="attbr")
        id_t = consts.tile([128, 128], f32, tag="ident")
        biasb_t = consts.tile([128, HC], f32, tag="biasb")
        dma(wl_t[:], wl)
        dma(wr_t[:], wr)
        dma(blp_t[:], blp)
        dma(brp_t[:], brp)
        dma(att2p_t[:], att2p)
        nc.vector.tensor_copy(att2p_r[:], att2p_t[:])
        nc.vector.tensor_copy(wl_r[:], wl_t[:])
        nc.vector.tensor_copy(wr_r[:], wr_t[:])
        dma(attb_t[:], attb)
        nc.vector.tensor_copy(attb_r[:], attb_t[:])
        dma(id_t[:], ident)
        dma(biasb_t[:], biasb)

        # ---------- load x, adj ----------
        xin = consts.tile([128, 8 * F], f32, tag="xin")       # x[b] tiles, [node128][f]
        for k in range(8):
            dma(xin[:, k * F:(k + 1) * F], xb[k * 128:(k + 1) * 128, :])
        xis_t = consts.tile([128, 2 * F], f32, tag="xis")
        for k in range(2):
            dma(xis_t[:, k * F:(k + 1) * F], xis[k * 128:(k + 1) * 128, :])
        adjm = consts.tile([128, 2 * N], f16, tag="adjm")     # [i128][ib*N + j]
        for ib in range(2):
            dma(adjm[:, ib * N:(ib + 1) * N], adjs[ib * 128:(ib + 1) * 128, :])

        # ---------- xT via PE transpose ----------
        xT = consts.tile([F, N], f32r, tag="xT")              # [f, node]
        for k in range(8):
            pt = pst.tile([128, 128], f32, tag="pt")
            nc.tensor.transpose(pt[:], xin[:, k * F:(k + 1) * F], id_t[:])
            nc.vector.tensor_copy(xT[:, k * 128:(k + 1) * 128], pt[:])
        xisT = consts.tile([F, NI], f32r, tag="xisT")
        for k in range(2):
            pt = pst.tile([128, 128], f32, tag="pt")
            nc.tensor.transpose(pt[:], xis_t[:, k * F:(k + 1) * F], id_t[:])
            nc.vector.tensor_copy(xisT[:, k * 128:(k + 1) * 128], pt[:])

        # ---------- projections:  xl2T = (x@W_l + b_l)^T stacked twice ----------
        xl2T = consts.tile([128, N], f16, tag="xl2T")         # fp16: rows 0:64 == 64:128
        xlT32 = consts.tile([HC, N], f32r, tag="xlT32")       # f32r copy for slT matmul
        xrT = consts.tile([HC, N], f32, tag="xrT")            # (x@W_r+b_r)^T, all nodes
        xrsT = consts.tile([HC, NI], f32, tag="xrsT")         # same, dest-row slice
        pj = psg.tile([HC, N], f32, tag="g")
        for half in range(2):
            s = slice(half * 512, (half + 1) * 512)
            nc.tensor.matmul(pj[:, s], wl_r[:], xT[:, s],
                             start=True, stop=True)
        nc.scalar.activation(xl2T[0:HC, :], pj[:], Act.Identity,
                             bias=blp_t[:, 0:1], scale=1.0)
        nc.scalar.activation(xl2T[HC:128, :], pj[:], Act.Identity,
                             bias=blp_t[:, 0:1], scale=1.0)
        nc.scalar.activation(xlT32[:], pj[:], Act.Identity,
                             bias=blp_t[:, 0:1], scale=1.0)
        pj2 = psg.tile([HC, N], f32, tag="g")
        for half in range(2):
            s = slice(half * 512, (half + 1) * 512)
            nc.tensor.matmul(pj2[:, s], wr_r[:], xT[:, s],
                             start=True, stop=True)
        nc.scalar.activation(xrT[:], pj2[:], Act.Identity,
                             bias=brp_t[:, 0:1], scale=1.0)
        pj3 = psg.tile([HC, NI], f32, tag="g")
        nc.tensor.matmul(pj3[:], wr_r[:], xisT[:],
                         start=True, stop=True)
        nc.scalar.activation(xrsT[:], pj3[:], Act.Identity,
                             bias=brp_t[:, 0:1], scale=1.0)

        # ---------- xrp: per-pair bias columns [xr[i0+2p] ; xr[i0+2p+1]] ----------
        xrp = consts.tile([128, NPAIR], f32, tag="xrp")
        ev = xrsT[:].rearrange("p (a two) -> p a two", two=2)
        nc.vector.tensor_copy(xrp[0:HC, :], ev[:, :, 0])
        nc.vector.tensor_copy(xrp[HC:128, :], ev[:, :, 1])

        # Deferred builds (emitted inside the main loop so the first contract
        # matmuls are not delayed): adjT before super-0 transposes, xr_mod
        # after super 0.
        xr_mod = consts.tile([128, 8 * 68], f16, tag="xrmod")
        adjT = consts.tile([128, 2 * N], f16, tag="adjT")
        id16 = consts.tile([128, 128], f16, tag="id16")
        nc.vector.tensor_copy(id16[:], id_t[:])

        def build_adjT(ib2):
            if True:
                for k in range(8):
                    pt = pst.tile([128, 128], f16, tag="pt", name="pt")
                    nc.tensor.transpose(pt[:], adjm[:, ib2 * N + k * 128: ib2 * N + (k + 1) * 128],
                                        id16[:])
                    nc.vector.tensor_copy(adjT[:, k * 256 + ib2 * 128: k * 256 + (ib2 + 1) * 128],
                                          pt[:])

        def build_xr_mod():
            # slT[h, j] = sum_hc att_blk[hc,h]*xl[hc,j];  esl = exp(0.2*sl)
            psl = psg.tile([H, N], f32, tag="g", name="psl")
            for half in range(2):
                s = slice(half * 512, (half + 1) * 512)
                nc.tensor.matmul(psl[:, s], attb_r[:],
                                 xlT32[:, s], start=True, stop=True)
            eslT = work.tile([H, N], f32, tag="eslT", name="eslT")
            nc.scalar.activation(eslT[:], psl[:], Act.Exp, scale=0.2)
            esl_nat = work.tile([128, 8 * H], f32, tag="eslnat", name="esl_nat")
            for k in range(8):
                pt = pst.tile([128, 128], f32, tag="pt", name="pt")
                nc.tensor.transpose(pt[:, 0:H], eslT[:, k * 128:(k + 1) * 128],
                                    id_t[0:H, 0:H])
                nc.vector.tensor_copy(esl_nat[:, k * H:(k + 1) * H], pt[:, 0:H])
                zcols = xr_mod[:].rearrange("p (k h c) -> p k h c", k=8, h=H)[:, k, :, 16]
                nc.vector.tensor_copy(zcols, pt[:, 0:H])
                pt2 = pst.tile([128, 128], f32, tag="pt", name="pt2")
                nc.tensor.transpose(pt2[:, 0:HC], xrT[:, k * 128:(k + 1) * 128],
                                    id_t[0:HC, 0:HC])
                xcols = xr_mod[:].rearrange("p (k h c) -> p k h c", k=8, h=H)[:, k, :, 0:16]
                srcx = pt2[:, 0:HC].rearrange("p (h c) -> p h c", h=H)
                rep = esl_nat[:, k * H:(k + 1) * H].rearrange("p (h one) -> p h one", one=1).broadcast_to([128, H, 16])
                nc.vector.tensor_tensor(xcols, srcx, rep, Alu.mult)
            if dbg_t:
                dma(dbg_t["d_eslT"], eslT[:])

        if dbg_t:
            dma(dbg_t["d_xl2T"], xl2T[:])
            dma(dbg_t["d_xrp"], xrp[:])

        # ---------- main streaming loop ----------
        st_t = [consts.tile([128, 8 * 512], f16, tag=f"stt{ib}", name=f"stt{ib}") for ib in range(2)]

        def aggregate(ib):
            out_f = outp.tile([128, HC], f32, tag="outf", name="outf")
            out_f2 = outp.tile([128, HC], f32, tag="outf2", name="outf2")
            for h in range(H):
                agg = psg.tile([128, 17], f32, tag="g", name="agg")
                for k in range(8):
                    lhs = st_t[ib][:].rearrange("p (k s a h) -> p k s a h",
                                                k=8, s=4, h=H)[:, k, :, :, h]
                    rhs = xr_mod[:, k * 68 + h * 17: k * 68 + (h + 1) * 17]
                    nc.tensor.matmul(agg[:], lhs, rhs,
                                     start=(k == 0), stop=(k == 7))
                rz = work.tile([128, 1], f32, tag="rz", name="rz")
                nc.vector.reciprocal(rz[:], agg[:, 16:17])
                nc.vector.tensor_scalar(out_f[:, h * 16:(h + 1) * 16],
                                        agg[:, 0:16], rz[:, 0:1], None, Alu.mult)
            nc.vector.tensor_add(out_f2[:], out_f[:], biasb_t[:])
            dma(out[ib * 128:(ib + 1) * 128, :], out_f2[:])

        for sup in range(NSUP):
            ib, s4 = sup // 4, sup % 4
            if sup == 1:
                build_xr_mod()
            scomp = scomp_pool.tile([128, N], f16, tag="scomp")
            for g in range(4):
                gps = psg.tile([128, N], f32, tag="g")
                for q4 in range(4):
                    p = sup * 16 + g * 4 + q4
                    rp = rp_pool.tile([128, N], f16, tag="rp")
                    if q4 >= GEN_ACT_MOD:
                        nc.scalar.activation(rp[:], xl2T[:], Act.Relu,
                                             bias=xrp[:, p:p + 1], scale=1.0)
                    else:
                        nc.vector.tensor_scalar(rp[:], xl2T[:],
                                                xrp[:, p:p + 1],
                                                0.0, Alu.add, Alu.max)
                    if dbg_t and p == 0:
                        dma(dbg_t["d_rp0"], rp[:])
                    for half in range(2):
                        s = slice(half * 512, (half + 1) * 512)
                        nc.tensor.matmul(gps[32 * q4:32 * q4 + 32, s],
                                         att2p_r[:],
                                         rp[:, s],
                                         start=True, stop=True,
                                         tile_position=(0, 32 * q4))
                ssb = ssb_pool.tile([128, N], f16, tag="ssb")
                nc.scalar.activation(ssb[:], gps[:], Act.Exp)
                if dbg_t and sup == 0 and g == 0:
                    dma(dbg_t["d_ssb0"], ssb[:])
                for c4 in range(4):
                    dma(scomp[g * 32 + c4 * 8: g * 32 + (c4 + 1) * 8, :],
                        ssb[32 * c4:32 * c4 + 8, :])
            if dbg_t and sup == 0:
                dma(dbg_t["d_scomp0"], scomp[:])
            if sup == 0:
                build_adjT(0)
            elif sup == 2:
                build_adjT(1)
            for k in range(8):
                pt = pst.tile([128, 128], f16, tag="pt", name="pt")
                nc.tensor.transpose(pt[:], scomp[:, k * 128:(k + 1) * 128], id16[:])
                msk = adjT[:, k * 256 + ib * 128 + s4 * 32:
                           k * 256 + ib * 128 + s4 * 32 + 32]
                mskr = msk.rearrange("p (a one) -> p a one", one=1).broadcast_to([128, 32, H])
                dstv = st_t[ib][:, k * 512 + s4 * 128: k * 512 + (s4 + 1) * 128]
                dstv = dstv.rearrange("p (a h) -> p a h", h=H)
                ptv = pt[:].rearrange("p (a h) -> p a h", h=H)
                nc.vector.tensor_tensor(dstv, ptv, mskr, Alu.mult)


        if dbg_t:
            dma(dbg_t["d_stt0"], st_t[0][:])
            dma(dbg_t["d_xrmod"], xr_mod[:])
            dma(dbg_t["d_adjT"], adjT[:])
        aggregate(0)
        aggregate(1)


def _get_program():
    if "nc" not in _CACHE:
        _CACHE["nc"] = _build_program()
    return _CACHE["nc"]


def kernel(x, adj, W_l, b_l, W_r, b_r, att, bias):
    global LAST_RESULTS
    from concourse.bass_utils import run_bass_kernel_spmd

    x = np.ascontiguousarray(np.asarray(x, dtype=np.float32))
    adj = np.ascontiguousarray(np.asarray(adj, dtype=np.float32))
    W_l = np.asarray(W_l, dtype=np.float32)
    b_l = np.asarray(b_l, dtype=np.float32)
    W_r = np.asarray(W_r, dtype=np.float32)
    b_r = np.asarray(b_r, dtype=np.float32)
    att = np.asarray(att, dtype=np.float32)
    bias = np.asarray(bias, dtype=np.float32)

    # host-side constant prep
    att2p = np.zeros((F, 32), np.float32)        # [0.8 * att2 | 0]
    for d in range(2):
        for h in range(H):
            att2p[d * HC + h * C:(d * HC + (h + 1) * C), d * H + h] = 0.8 * att[h]
    attb = np.zeros((HC, H), np.float32)         # att_blk
    for h in range(H):
        attb[h * C:(h + 1) * C, h] = att[h]
    identity = np.eye(128, dtype=np.float32)
    biasb = np.broadcast_to(bias, (128, HC)).copy()
    blp = b_l.reshape(HC, 1).copy()
    brp = b_r.reshape(HC, 1).copy()

    in_maps = []
    for core in range(NCORES):
        b, blk = core // 4, core % 4
        i0 = blk * NI
        adjs = adj[b, i0:i0 + NI, :].copy()
        adjs[np.arange(NI), i0 + np.arange(NI)] = 1.0   # self loops
        adjs = adjs.astype(np.float16)
        in_maps.append({
            "xb": x[b], "xis": x[b, i0:i0 + NI].copy(), "adjs": adjs,
            "wl": W_l, "wr": W_r, "blp": blp, "brp": brp,
            "att2p": att2p, "attb": attb, "ident": identity, "biasb": biasb,
        })

    nc = _get_program()
    res = run_bass_kernel_spmd(nc, in_maps, core_ids=list(range(NCORES)))
    LAST_RESULTS = res
    outp = np.zeros((B, N, HC), np.float32)
    for core in range(NCORES):
        b, blk = core // 4, core % 4
        outp[b, blk * NI:(blk + 1) * NI, :] = res.results[core]["out"]
    return outp



# revision 8
# speedup vs baseline: 1.9731x; 1.9731x over previous
# DenseGATv2Conv Trainium2 kernel.
#
# Math (per batch b):
#   xl = x @ W_l + b_l ; xr = x @ W_r + b_r            [N, H*C]
#   alpha[i,j,h] = sum_c att[h,c] * leaky_relu(xl[j,hc] + xr[i,hc], 0.2)
#   S = softmax_j(alpha masked by adj(+self loops))
#   out[i,hc] = sum_j S[i,j,h] * xr[j,hc] + bias
#
# Key identity used on device:
#   leaky_relu(z) = 0.2*z + 0.8*relu(z)
#   alpha[i,j,h] = 0.2*sl[j,h] + 0.2*sr[i,h] + 0.8*sum_c att[h,c]*relu(xl[j,hc]+xr[i,hc])
# where sl = xl @ att_blk, sr = xr @ att_blk are rank-1 in the (i,j) plane.
# In softmax over j the exp(0.2*sr[i,h]) factor cancels; exp(0.2*sl[j,h]) is
# folded multiplicatively into the aggregation operand.
#
# Device scheme (v2): for each pair of destination rows (2 per "pair", 16
# pairs = 32 dest rows per "super"), DVE/Act/Pool build
#   rp[(d,hc), j] = relu(xl[j,hc] + xr[i_d,hc])         [128, N] f16
# Then the score contraction runs with rp as the matmul STATIONARY operand
# and a tiny [128, 8] att operand moving, so the result lands in PSUM
# already transposed: pst[j, (pair,d,h)].  The adjacency mask is folded in
# as an extra accumulating matmul that adds -30 to masked entries
# (adjm30 = 30*(adj-1) in {0,-30} f16, stationary; selection matrix E30
# moving), so a single exp() per super yields masked scores directly
# (exp(a-30) underflows f16 to 0).  The softmax denominator is computed in
# the aggregation matmul via an extra ones-like column (esl factor).
#
# Sharding: 8 cores = (batch b in 0..1) x (4 blocks of 256 destination rows).

import numpy as np

B, N, F, H, C = 2, 1024, 128, 4, 16
HC = H * C
NCORES = 8
NI = 256          # destination rows per core
NPAIR = NI // 2   # 128 pairs of destination rows
NSUP = 8          # supers of 16 pairs (32 dest rows) each
NEG = 0.2
MASKVAL = 30.0

_CACHE = {}
LAST_RESULTS = None

# engine assignment for the 16 rp builds of each super:
# 'd' = DVE tensor_scalar, 'a' = Act activation, 'p' = Pool tensor_scalar
RP_SCHED = "ddddddddddddaapp"

# blob column layout (f32 [128, BLOB_COLS]):
#   xb      [128, 8*128]   x[b] nodes, node k*128+p at cols k*128..k*128+128? no:
#                          xin[p, k*128+f] = x[k*128+p, f]
#   xis     [128, 2*128]   dest-row slice, same layout
#   wl      [128, 64]
#   wr      [128, 64]
#   ident   [128, 128]
#   biasb   [128, 64]
#   att2p8  [128, 8]
#   blp2    [128, 1]       b_l tiled twice
#   brp2    [128, 1]       b_r tiled twice
#   attb    [128, 4]       att_blk (rows 64:128 zero)
#   e30     [128, 128]     selection matrix for mask matmul
_OFF = {}
_c = 0
for _nm, _w in [("xb", 8 * F), ("xis", 2 * F), ("wl", HC), ("wr", HC),
                ("ident", 128), ("biasb", HC), ("att2p8", 8), ("blp2", 1),
                ("brp2", 1), ("attb", H), ("e30", 128)]:
    _OFF[_nm] = _c
    _c += _w
BLOB_COLS = _c


def _build_program(debug=False):
    import concourse.bass as bass
    import concourse.mybir as mybir
    import concourse.tile as tile
    from concourse import bacc

    f32 = mybir.dt.float32
    f32r = mybir.dt.float32r
    f16 = mybir.dt.float16

    nc = bacc.Bacc(
        "TRN2",
        target_bir_lowering=False,
        debug=False,
        enable_asserts=False,
        num_devices=NCORES,
    )

    # ---- DRAM I/O ----
    blob = nc.dram_tensor("blob", [128, BLOB_COLS], f32, kind="ExternalInput").ap()
    adjs = nc.dram_tensor("adjs", [128, 2 * N], f16, kind="ExternalInput").ap()
    out = nc.dram_tensor("out", [NI, HC], f32, kind="ExternalOutput").ap()

    with tile.TileContext(nc) as tc:
        _body(tc, nc, mybir, bass, f32, f32r, f16, blob, adjs, out)

    nc.compile()
    return nc


def _body(tc, nc, mybir, bass, f32, f32r, f16, blob, adjs, out):
    from contextlib import ExitStack
    Alu = mybir.AluOpType
    Act = mybir.ActivationFunctionType
    ctx = ExitStack()
    with ctx:
        consts = ctx.enter_context(tc.tile_pool(name="consts", bufs=1))
        work = ctx.enter_context(tc.tile_pool(name="work", bufs=1))
        rp_pool = ctx.enter_context(tc.tile_pool(name="rp", bufs=2))
        outp = ctx.enter_context(tc.tile_pool(name="outp", bufs=2))
        psg = ctx.enter_context(tc.tile_pool(name="psg", bufs=1, space="PSUM"))
        pss = ctx.enter_context(tc.tile_pool(name="pss", bufs=2, space="PSUM"))
        pst = ctx.enter_context(tc.tile_pool(name="pst", bufs=2, space="PSUM"))

        dma = nc.sync.dma_start

        # ---------- load inputs ----------
        blob_t = consts.tile([128, BLOB_COLS], f32, tag="blob")
        dma(blob_t[:], blob)
        adjm = consts.tile([128, 2 * N], f16, tag="adjm")   # 30*(adj-1), [i128][ib*N+j]
        dma(adjm[:], adjs)

        def bv(nm, w):
            return blob_t[:, _OFF[nm]:_OFF[nm] + w]

        xin = bv("xb", 8 * F)
        xis_t = bv("xis", 2 * F)
        id_t = bv("ident", 128)
        biasb_t = bv("biasb", HC)
        blp_t = blob_t[0:HC, _OFF["blp2"]:_OFF["blp2"] + 1]
        brp_t = blob_t[0:HC, _OFF["brp2"]:_OFF["brp2"] + 1]

        # f32r / f16 copies of small constants
        wl_r = consts.tile([F, HC], f32r, tag="wlr")
        wr_r = consts.tile([F, HC], f32r, tag="wrr")
        attb_r = consts.tile([HC, H], f32r, tag="attbr")
        att8_r = consts.tile([F, 8], f16, tag="att8r")
        e30_16 = consts.tile([128, 128], f16, tag="e30")
        nc.vector.tensor_copy(wl_r[:], bv("wl", HC))
        nc.vector.tensor_copy(wr_r[:], bv("wr", HC))
        nc.vector.tensor_copy(attb_r[:], blob_t[0:HC, _OFF["attb"]:_OFF["attb"] + H])
        nc.vector.tensor_copy(att8_r[:], bv("att2p8", 8))
        nc.gpsimd.tensor_copy(e30_16[:], bv("e30", 128))

        # ---------- xT via PE transpose ----------
        xT = consts.tile([F, N], f32r, tag="xT")              # [f, node]
        for k in range(8):
            pt = pst.tile([128, 128], f32, tag="pt")
            nc.tensor.transpose(pt[:], xin[:, k * F:(k + 1) * F], id_t)
            if k % 2 == 0:
                nc.vector.tensor_copy(xT[:, k * 128:(k + 1) * 128], pt[:])
            else:
                nc.scalar.copy(xT[:, k * 128:(k + 1) * 128], pt[:])
        xisT = consts.tile([F, NI], f32r, tag="xisT")
        for k in range(2):
            pt = pst.tile([128, 128], f32, tag="pt")
            nc.tensor.transpose(pt[:], xis_t[:, k * F:(k + 1) * F], id_t)
            if k % 2 == 0:
                nc.vector.tensor_copy(xisT[:, k * 128:(k + 1) * 128], pt[:])
            else:
                nc.scalar.copy(xisT[:, k * 128:(k + 1) * 128], pt[:])

        # ---------- projections ----------
        # xl2T = (x@W_l + b_l)^T stacked twice (f16), xlT32 f32r for slT
        xl2T = consts.tile([128, N], f16, tag="xl2T")
        xlT32 = consts.tile([HC, N], f32r, tag="xlT32")
        xrT = consts.tile([HC, N], f32, tag="xrT")
        xrsT = consts.tile([HC, NI], f32, tag="xrsT")
        pj = psg.tile([HC, N], f32, tag="g")
        for half in range(2):
            s = slice(half * 512, (half + 1) * 512)
            nc.tensor.matmul(pj[:, s], wl_r[:], xT[:, s], start=True, stop=True)
        nc.scalar.activation(xl2T[0:HC, :], pj[:], Act.Identity,
                             bias=blp_t, scale=1.0)
        nc.vector.tensor_copy(xl2T[HC:128, :], xl2T[0:HC, :])
        nc.scalar.activation(xlT32[:], pj[:], Act.Identity,
                             bias=blp_t, scale=1.0)
        pj2 = psg.tile([HC, N], f32, tag="g")
        for half in range(2):
            s = slice(half * 512, (half + 1) * 512)
            nc.tensor.matmul(pj2[:, s], wr_r[:], xT[:, s], start=True, stop=True)
        nc.scalar.activation(xrT[:], pj2[:], Act.Identity,
                             bias=brp_t, scale=1.0)
        pj3 = psg.tile([HC, NI], f32, tag="g")
        nc.tensor.matmul(pj3[:], wr_r[:], xisT[:], start=True, stop=True)
        nc.scalar.activation(xrsT[:], pj3[:], Act.Identity,
                             bias=brp_t, scale=1.0)

        # ---------- xrp: per-pair bias columns [xr[i0+2p] ; xr[i0+2p+1]] ----------
        xrp = consts.tile([128, NPAIR], f32, tag="xrp")
        ev = xrsT[:].rearrange("p (a two) -> p a two", two=2)
        nc.vector.tensor_copy(xrp[0:HC, :], ev[:, :, 0])
        nc.vector.tensor_copy(xrp[HC:128, :], ev[:, :, 1])

        # ---------- xr_mod: aggregation moving operand ----------
        # xr_mod[j, k*68 + h*17 + c] = xr[j,hc]*esl[j,h] (c<16); col c=16: esl[j,h]
        xr_mod = consts.tile([128, 8 * 68], f16, tag="xrmod")

        def build_xr_mod():
            psl = psg.tile([H, N], f32, tag="g", name="psl")
            for half in range(2):
                s = slice(half * 512, (half + 1) * 512)
                nc.tensor.matmul(psl[:, s], attb_r[:], xlT32[:, s],
                                 start=True, stop=True)
            eslT = work.tile([H, N], f32, tag="eslT", name="eslT")
            nc.scalar.activation(eslT[:], psl[:], Act.Exp, scale=NEG)
            esl_nat = work.tile([128, 8 * H], f32, tag="eslnat", name="esl_nat")
            for k in range(8):
                pt = pst.tile([128, 128], f32, tag="pt", name="pt")
                nc.tensor.transpose(pt[:, 0:H], eslT[:, k * 128:(k + 1) * 128],
                                    id_t[0:H, 0:H])
                nc.vector.tensor_copy(esl_nat[:, k * H:(k + 1) * H], pt[:, 0:H])
                zcols = xr_mod[:].rearrange("p (k h c) -> p k h c", k=8, h=H)[:, k, :, 16]
                nc.vector.tensor_copy(zcols, pt[:, 0:H])
                pt2 = pst.tile([128, 128], f32, tag="pt", name="pt2")
                nc.tensor.transpose(pt2[:, 0:HC], xrT[:, k * 128:(k + 1) * 128],
                                    id_t[0:HC, 0:HC])
                xcols = xr_mod[:].rearrange("p (k h c) -> p k h c", k=8, h=H)[:, k, :, 0:16]
                srcx = pt2[:, 0:HC].rearrange("p (h c) -> p h c", h=H)
                rep = esl_nat[:, k * H:(k + 1) * H].rearrange(
                    "p (h one) -> p h one", one=1).broadcast_to([128, H, 16])
                nc.vector.tensor_tensor(xcols, srcx, rep, Alu.mult)

        # ---------- main streaming loop ----------
        # st_all[j, k*1024 + s*128 + a*4 + h], a = local dest (2*p+d) in super s
        # (k-major so the aggregation lhsT slice has ONE contiguous free dim)
        st_all = consts.tile([128, NSUP * N], f16, tag="stall")
        stv = st_all[:].rearrange("p (k s a h) -> p k s a h", k=8, s=NSUP, h=H)

        for sup in range(NSUP):
            ib, s4 = sup // 4, sup % 4
            if sup == 1:
                build_xr_mod()
            rp = rp_pool.tile([128, 16, 1024], f16, tag="rp")
            for p in range(16):
                gp = sup * 16 + p
                kind = RP_SCHED[p]
                if kind == "a":
                    nc.scalar.activation(rp[:, p, :], xl2T[:], Act.Relu,
                                         bias=xrp[:, gp:gp + 1], scale=1.0)
                elif kind == "p":
                    nc.gpsimd.tensor_scalar(rp[:, p, :], xl2T[:],
                                            xrp[:, gp:gp + 1],
                                            0.0, Alu.add, Alu.max)
                else:
                    nc.vector.tensor_scalar(rp[:, p, :], xl2T[:],
                                            xrp[:, gp:gp + 1],
                                            0.0, Alu.add, Alu.max)
            ps = pss.tile([128, 8, 128], f32, tag="sc")
            for k in range(8):
                # mask matmul: adds 30*(adj-1) (0 or -30) to every (j, a, h)
                nc.tensor.matmul(ps[:, k, :],
                                 adjm[s4 * 32:(s4 + 1) * 32,
                                      ib * N + k * 128: ib * N + (k + 1) * 128],
                                 e30_16[s4 * 32:(s4 + 1) * 32, :],
                                 start=True, stop=False, skip_group_check=True,
                                 tile_position=(s4 * 32, 0))
                for p in range(16):
                    nc.tensor.matmul(ps[:, k, p * 8:(p + 1) * 8],
                                     rp[:, p, k * 128:(k + 1) * 128],
                                     att8_r[:],
                                     start=False, stop=(p == 15),
                                     skip_group_check=True)
            nc.scalar.activation(stv[:, :, sup, :, :],
                                 ps[:].rearrange("p k (a h) -> p k a h", h=H), Act.Exp)

        # ---------- aggregation ----------
        for ib in range(2):
            out_f = outp.tile([128, HC], f32, tag="outf", name="outf")
            out_f2 = outp.tile([128, HC], f32, tag="outf2", name="outf2")
            for h in range(H):
                agg = psg.tile([128, 17], f32, tag="g", name="agg")
                for k in range(8):
                    lhs = stv[:, k, ib * 4:(ib + 1) * 4, :, h]
                    rhs = xr_mod[:, k * 68 + h * 17: k * 68 + (h + 1) * 17]
                    nc.tensor.matmul(agg[:], lhs, rhs,
                                     start=(k == 0), stop=(k == 7))
                rz = work.tile([128, 1], f32, tag="rz", name="rz")
                nc.vector.reciprocal(rz[:], agg[:, 16:17])
                nc.vector.tensor_scalar(out_f[:, h * 16:(h + 1) * 16],
                                        agg[:, 0:16], rz[:, 0:1], None, Alu.mult)
            nc.vector.tensor_add(out_f2[:], out_f[:], biasb_t)
            dma(out[ib * 128:(ib + 1) * 128, :], out_f2[:])


def _get_program():
    if "nc" not in _CACHE:
        _CACHE["nc"] = _build_program()
    return _CACHE["nc"]


def kernel(x, adj, W_l, b_l, W_r, b_r, att, bias):
    global LAST_RESULTS
    from concourse.bass_utils import run_bass_kernel_spmd

    x = np.ascontiguousarray(np.asarray(x, dtype=np.float32))
    adj = np.ascontiguousarray(np.asarray(adj, dtype=np.float32))
    W_l = np.asarray(W_l, dtype=np.float32)
    b_l = np.asarray(b_l, dtype=np.float32)
    W_r = np.asarray(W_r, dtype=np.float32)
    b_r = np.asarray(b_r, dtype=np.float32)
    att = np.asarray(att, dtype=np.float32)
    bias = np.asarray(bias, dtype=np.float32)

    # host-side constant prep
    att2p8 = np.zeros((F, 8), np.float32)        # 0.8 * att, block diagonal x2
    for d in range(2):
        for h in range(H):
            att2p8[d * HC + h * C:(d * HC + (h + 1) * C), d * H + h] = 0.8 * att[h]
    attb = np.zeros((128, H), np.float32)        # att_blk (rows 0:64)
    for h in range(H):
        attb[h * C:(h + 1) * C, h] = att[h]
    e30 = np.zeros((128, 128), np.float32)       # E30[r, a*4+h] = (r%32 == a)
    r = np.arange(128)
    for a in range(32):
        for h in range(H):
            e30[r[r % 32 == a], a * 4 + h] = 1.0

    blob = np.zeros((128, BLOB_COLS), np.float32)

    def put(nm, arr):
        w = arr.shape[1]
        blob[:arr.shape[0], _OFF[nm]:_OFF[nm] + w] = arr

    put("wl", W_l)
    put("wr", W_r)
    put("ident", np.eye(128, dtype=np.float32))
    put("biasb", np.broadcast_to(bias, (128, HC)))
    put("att2p8", att2p8)
    put("blp2", np.tile(b_l, 2).reshape(128, 1))
    put("brp2", np.tile(b_r, 2).reshape(128, 1))
    put("attb", attb)
    put("e30", e30)

    in_maps = []
    for core in range(NCORES):
        b, blk = core // 4, core % 4
        i0 = blk * NI
        cblob = blob.copy()
        # xin[p, k*128+f] = x[b, k*128+p, f]
        cblob[:, _OFF["xb"]:_OFF["xb"] + 8 * F] = (
            x[b].reshape(8, 128, F).transpose(1, 0, 2).reshape(128, 8 * F))
        cblob[:, _OFF["xis"]:_OFF["xis"] + 2 * F] = (
            x[b, i0:i0 + NI].reshape(2, 128, F).transpose(1, 0, 2).reshape(128, 2 * F))
        adjsl = adj[b, i0:i0 + NI, :].copy()
        adjsl[np.arange(NI), i0 + np.arange(NI)] = 1.0   # self loops
        adjm30 = (MASKVAL * (adjsl - 1.0)).astype(np.float16)
        # adjm[p, ib*N + j] = adjm30[ib*128 + p, j]
        adjm = adjm30.reshape(2, 128, N).transpose(1, 0, 2).reshape(128, 2 * N).copy()
        in_maps.append({"blob": cblob, "adjs": adjm})

    nc = _get_program()
    res = run_bass_kernel_spmd(nc, in_maps, core_ids=list(range(NCORES)))
    LAST_RESULTS = res
    outp = np.zeros((B, N, HC), np.float32)
    for core in range(NCORES):
        b, blk = core // 4, core % 4
        outp[b, blk * NI:(blk + 1) * NI, :] = res.results[core]["out"]
    return outp


# revision 11
# speedup vs baseline: 2.1689x; 1.0992x over previous
# DenseGATv2Conv Trainium2 kernel.
#
# Math (per batch b):
#   xl = x @ W_l + b_l ; xr = x @ W_r + b_r            [N, H*C]
#   alpha[i,j,h] = sum_c att[h,c] * leaky_relu(xl[j,hc] + xr[i,hc], 0.2)
#   S = softmax_j(alpha masked by adj(+self loops))
#   out[i,hc] = sum_j S[i,j,h] * xr[j,hc] + bias
#
# Key identity used on device:
#   leaky_relu(z) = 0.2*z + 0.8*relu(z)
#   alpha[i,j,h] = 0.2*sl[j,h] + 0.2*sr[i,h] + 0.8*sum_c att[h,c]*relu(xl[j,hc]+xr[i,hc])
# where sl = xl @ att_blk, sr = xr @ att_blk are rank-1 in the (i,j) plane.
# In softmax over j the exp(0.2*sr[i,h]) factor cancels; exp(0.2*sl[j,h]) is
# folded multiplicatively into the aggregation operand.
#
# Device scheme (v2): for each pair of destination rows (2 per "pair", 16
# pairs = 32 dest rows per "super"), DVE/Act/Pool build
#   rp[(d,hc), j] = relu(xl[j,hc] + xr[i_d,hc])         [128, N] f16
# Then the score contraction runs with rp as the matmul STATIONARY operand
# and a tiny [128, 8] att operand moving, so the result lands in PSUM
# already transposed: pst[j, (pair,d,h)].  The adjacency mask is folded in
# as an extra accumulating matmul that adds -30 to masked entries
# (adjm30 = 30*(adj-1) in {0,-30} f16, stationary; selection matrix E30
# moving), so a single exp() per super yields masked scores directly
# (exp(a-30) underflows f16 to 0).  The softmax denominator is computed in
# the aggregation matmul via an extra ones-like column (esl factor).
#
# Sharding: 8 cores = (batch b in 0..1) x (4 blocks of 256 destination rows).

import numpy as np

B, N, F, H, C = 2, 1024, 128, 4, 16
HC = H * C
NCORES = 8
NI = 256          # destination rows per core
NPAIR = NI // 2   # 128 pairs of destination rows
NSUP = 8          # supers of 16 pairs (32 dest rows) each
NEG = 0.2
MASKVAL = 30.0

_CACHE = {}
LAST_RESULTS = None

# engine assignment for the 16 rp builds of each super:
# 'd' = DVE tensor_scalar, 'a' = Act activation, 'p' = Pool tensor_scalar
RP_SCHED = "ddddddddddddaapp"

# blob column layout (f32 [128, BLOB_COLS]):
#   xb      [128, 8*128]   x[b] nodes, node k*128+p at cols k*128..k*128+128? no:
#                          xin[p, k*128+f] = x[k*128+p, f]
#   xis     [128, 2*128]   dest-row slice, same layout
#   wl      [128, 64]
#   wr      [128, 64]
#   ident   [128, 128]
#   biasb   [128, 64]
#   att2p8  [128, 8]
#   blp2    [128, 1]       b_l tiled twice
#   brp2    [128, 1]       b_r tiled twice
#   attb    [128, 4]       att_blk (rows 64:128 zero)
#   e30     [128, 128]     selection matrix for mask matmul
_OFF = {}
_c = 0
for _nm, _w in [("xis", 2 * F), ("wl", HC), ("wr", HC), ("wrab", HC + H),
                ("ident", 128), ("biasb", HC), ("att2p8", 8), ("blp2", 1),
                ("brp2", 1), ("e30", 128), ("xb", 8 * F)]:
    _OFF[_nm] = _c
    _c += _w
HEAD_COLS = _OFF["xb"]
BLOB_COLS = _c


def _build_program(debug=False):
    import concourse.bass as bass
    import concourse.mybir as mybir
    import concourse.tile as tile
    from concourse import bacc

    f32 = mybir.dt.float32
    f32r = mybir.dt.float32r
    f16 = mybir.dt.float16

    nc = bacc.Bacc(
        "TRN2",
        target_bir_lowering=False,
        debug=False,
        enable_asserts=False,
        num_devices=NCORES,
    )

    # ---- DRAM I/O ----
    blob = nc.dram_tensor("blob", [128, BLOB_COLS], f32, kind="ExternalInput").ap()
    adjs = nc.dram_tensor("adjs", [128, 2 * N], f16, kind="ExternalInput").ap()
    out = nc.dram_tensor("out", [NI, HC], f32, kind="ExternalOutput").ap()

    with tile.TileContext(nc) as tc:
        _body(tc, nc, mybir, bass, f32, f32r, f16, blob, adjs, out)

    nc.compile()
    return nc


def _body(tc, nc, mybir, bass, f32, f32r, f16, blob, adjs, out):
    from contextlib import ExitStack
    Alu = mybir.AluOpType
    Act = mybir.ActivationFunctionType
    ctx = ExitStack()
    with ctx:
        consts = ctx.enter_context(tc.tile_pool(name="consts", bufs=1))
        work = ctx.enter_context(tc.tile_pool(name="work", bufs=1))
        rp_pool = ctx.enter_context(tc.tile_pool(name="rp", bufs=2))
        outp = ctx.enter_context(tc.tile_pool(name="outp", bufs=2))
        psg = ctx.enter_context(tc.tile_pool(name="psg", bufs=1, space="PSUM"))
        pss = ctx.enter_context(tc.tile_pool(name="pss", bufs=2, space="PSUM"))
        pst = ctx.enter_context(tc.tile_pool(name="pst", bufs=2, space="PSUM"))

        dma = nc.sync.dma_start

        # ---------- load inputs ----------
        blob_t = consts.tile([128, BLOB_COLS], f32, tag="blob")
        dma(blob_t[:, 0:HEAD_COLS], blob[:, 0:HEAD_COLS])
        dma(blob_t[:, HEAD_COLS:], blob[:, HEAD_COLS:])
        adjm = consts.tile([128, 2 * N], f16, tag="adjm")   # 30*(adj-1), [i128][ib*N+j]
        dma(adjm[:], adjs)

        def bv(nm, w):
            return blob_t[:, _OFF[nm]:_OFF[nm] + w]

        xin = bv("xb", 8 * F)
        xis_t = bv("xis", 2 * F)
        id_t = bv("ident", 128)
        biasb_t = bv("biasb", HC)
        blp_t = blob_t[0:HC, _OFF["blp2"]:_OFF["blp2"] + 1]
        brp_t = blob_t[0:HC, _OFF["brp2"]:_OFF["brp2"] + 1]

        # f32r / f16 copies of small constants
        wl_r = consts.tile([F, HC], f32r, tag="wlr")
        wr_r = consts.tile([F, HC], f32r, tag="wrr")
        wrab_r = consts.tile([F, HC + H], f32r, tag="wrabr")
        att8_r = consts.tile([F, 8], f16, tag="att8r")
        e30_16 = consts.tile([128, 128], f16, tag="e30")
        nc.vector.tensor_copy(wl_r[:], bv("wl", HC))
        nc.vector.tensor_copy(wr_r[:], bv("wr", HC))
        nc.gpsimd.tensor_copy(wrab_r[:], bv("wrab", HC + H))
        nc.vector.tensor_copy(att8_r[:], bv("att2p8", 8))
        nc.gpsimd.tensor_copy(e30_16[:], bv("e30", 128))

        # ---------- xisT / xrsT / xrp (critical path to rp) ----------
        xisT = consts.tile([F, NI], f32r, tag="xisT")
        for k in range(2):
            pt = pst.tile([128, 128], f32, tag="pt")
            nc.tensor.transpose(pt[:], xis_t[:, k * F:(k + 1) * F], id_t)
            nc.vector.tensor_copy(xisT[:, k * 128:(k + 1) * 128], pt[:])
        xrsT = consts.tile([HC, NI], f32, tag="xrsT")
        g3 = psg.tile([128, N], f32, tag="g")
        pj3 = g3[0:HC, 0:NI]
        nc.tensor.matmul(pj3, wr_r[:], xisT[:], start=True, stop=True)
        nc.scalar.activation(xrsT[:], pj3, Act.Identity,
                             bias=brp_t, scale=1.0)
        xrp = consts.tile([128, NPAIR], f32, tag="xrp")
        ev = xrsT[:].rearrange("p (a two) -> p a two", two=2)
        nc.vector.tensor_copy(xrp[0:HC, :], ev[:, :, 0])
        nc.vector.tensor_copy(xrp[HC:128, :], ev[:, :, 1])

        # ---------- xT + xl2T ----------
        xT = consts.tile([F, N], f32r, tag="xT")              # [f, node]
        for k in range(8):
            pt = pst.tile([128, 128], f32, tag="pt")
            nc.tensor.transpose(pt[:], xin[:, k * F:(k + 1) * F], id_t)
            if k % 2 == 0:
                nc.vector.tensor_copy(xT[:, k * 128:(k + 1) * 128], pt[:])
            else:
                nc.scalar.copy(xT[:, k * 128:(k + 1) * 128], pt[:])
        xl2T = consts.tile([128, N], f16, tag="xl2T")
        gp = psg.tile([128, N], f32, tag="g")
        pj = gp[0:HC, :]
        for half in range(2):
            s = slice(half * 512, (half + 1) * 512)
            nc.tensor.matmul(pj[:, s], wl_r[:], xT[:, s], start=True, stop=True)
        nc.scalar.activation(xl2T[0:HC, :], pj, Act.Identity,
                             bias=blp_t, scale=1.0)
        nc.vector.tensor_copy(xl2T[HC:128, :], xl2T[0:HC, :])

        # ---------- xr_mod: aggregation moving operand (natural layout) ----------
        # xr_mod[j, k*68 + h*17 + c] = xr_nob[j,hc]*esl[j,h] (c<16); c=16: esl[j,h]
        # xr_nob excludes b_r (folded into the output bias since sum_j Sbar = 1);
        # esl = exp(0.2 * x @ (W_l @ att_blk)) (the b_l@att_blk factor cancels).
        xr_mod = consts.tile([128, 8 * 68], f16, tag="xrmod")
        xrmv = xr_mod[:].rearrange("p (k h c) -> p k h c", k=8, h=H)

        def build_xr_mod_k(k):
            pkt = pst.tile([128, 128], f32, tag="pt", name="pk")
            pk = pkt[:, 0:HC + H]
            nc.tensor.matmul(pk, xT[:, k * 128:(k + 1) * 128], wrab_r[:],
                             start=True, stop=True)
            esl4 = work.tile([128, 8 * H], f16, tag="esl4", name="esl4")
            xr16 = work.tile([128, 8 * HC], f16, tag="xr16", name="xr16")
            nc.scalar.activation(esl4[:, k * H:(k + 1) * H],
                                 pkt[:, HC:HC + H], Act.Exp, scale=NEG)
            nc.scalar.copy(xr16[:, k * HC:(k + 1) * HC], pkt[:, 0:HC])
            nc.gpsimd.tensor_copy(xrmv[:, k, :, 16], esl4[:, k * H:(k + 1) * H])
            rep = esl4[:, k * H:(k + 1) * H].rearrange(
                "p (h one) -> p h one", one=1).broadcast_to([128, H, 16])
            srcx = xr16[:, k * HC:(k + 1) * HC].rearrange("p (h c) -> p h c", h=H)
            nc.gpsimd.tensor_tensor(xrmv[:, k, :, 0:16], srcx, rep, Alu.mult)

        # ---------- main streaming loop ----------
        # st_all[j, k*1024 + s*128 + a*4 + h], a = local dest (2*p+d) in super s
        # (k-major so the aggregation lhsT slice has ONE contiguous free dim)
        st_all = consts.tile([128, NSUP * N], f16, tag="stall")
        stv = st_all[:].rearrange("p (k s a h) -> p k s a h", k=8, s=NSUP, h=H)

        def super_iter(sup):
            ib, s4 = sup // 4, sup % 4
            if sup < 4:
                build_xr_mod_k(2 * sup)
                build_xr_mod_k(2 * sup + 1)
            rp = rp_pool.tile([128, 16, 1024], f16, tag="rp")
            for p in range(16):
                gp = sup * 16 + p
                kind = RP_SCHED[p]
                if kind == "a":
                    nc.scalar.activation(rp[:, p, :], xl2T[:], Act.Relu,
                                         bias=xrp[:, gp:gp + 1], scale=1.0)
                elif kind == "p":
                    nc.gpsimd.tensor_scalar(rp[:, p, :], xl2T[:],
                                            xrp[:, gp:gp + 1],
                                            0.0, Alu.add, Alu.max)
                else:
                    nc.vector.tensor_scalar(rp[:, p, :], xl2T[:],
                                            xrp[:, gp:gp + 1],
                                            0.0, Alu.add, Alu.max)
            ps = pss.tile([128, 8, 128], f32, tag="sc")
            for k in range(8):
                # mask matmul: adds 30*(adj-1) (0 or -30) to every (j, a, h)
                nc.tensor.matmul(ps[:, k, :],
                                 adjm[s4 * 32:(s4 + 1) * 32,
                                      ib * N + k * 128: ib * N + (k + 1) * 128],
                                 e30_16[s4 * 32:(s4 + 1) * 32, :],
                                 start=True, stop=False, skip_group_check=True,
                                 tile_position=(s4 * 32, 0))
                for p in range(16):
                    nc.tensor.matmul(ps[:, k, p * 8:(p + 1) * 8],
                                     rp[:, p, k * 128:(k + 1) * 128],
                                     att8_r[:],
                                     start=False, stop=(p == 15),
                                     skip_group_check=True)
            nc.scalar.activation(stv[:, :, sup, :, :],
                                 ps[:].rearrange("p k (a h) -> p k a h", h=H), Act.Exp)

        # ---------- aggregation ----------
        def aggregate(ib):
            out_f = outp.tile([128, HC], f32, tag="outf", name="outf")
            out_f2 = outp.tile([128, HC], f32, tag="outf2", name="outf2")
            for h in range(H):
                ga = psg.tile([128, N], f32, tag="g", name="agg")
                agg = ga[:, 0:17]
                for k in range(8):
                    lhs = stv[:, k, ib * 4:(ib + 1) * 4, :, h]
                    rhs = xr_mod[:, k * 68 + h * 17: k * 68 + (h + 1) * 17]
                    nc.tensor.matmul(agg, lhs, rhs,
                                     start=(k == 0), stop=(k == 7))
                rz = work.tile([128, 1], f32, tag="rz", name="rz")
                nc.vector.reciprocal(rz[:], ga[:, 16:17])
                nc.vector.tensor_scalar(out_f[:, h * 16:(h + 1) * 16],
                                        ga[:, 0:16], rz[:, 0:1], None, Alu.mult)
            nc.vector.tensor_add(out_f2[:], out_f[:], biasb_t)
            dma(out[ib * 128:(ib + 1) * 128, :], out_f2[:])

        for sup in range(NSUP):
            super_iter(sup)
            if sup == 5:
                aggregate(0)
        aggregate(1)


def _get_program():
    if "nc" not in _CACHE:
        _CACHE["nc"] = _build_program()
    return _CACHE["nc"]


def kernel(x, adj, W_l, b_l, W_r, b_r, att, bias):
    global LAST_RESULTS
    from concourse.bass_utils import run_bass_kernel_spmd

    x = np.ascontiguousarray(np.asarray(x, dtype=np.float32))
    adj = np.ascontiguousarray(np.asarray(adj, dtype=np.float32))
    W_l = np.asarray(W_l, dtype=np.float32)
    b_l = np.asarray(b_l, dtype=np.float32)
    W_r = np.asarray(W_r, dtype=np.float32)
    b_r = np.asarray(b_r, dtype=np.float32)
    att = np.asarray(att, dtype=np.float32)
    bias = np.asarray(bias, dtype=np.float32)

    # host-side constant prep
    att2p8 = np.zeros((F, 8), np.float32)        # 0.8 * att, block diagonal x2
    for d in range(2):
        for h in range(H):
            att2p8[d * HC + h * C:(d * HC + (h + 1) * C), d * H + h] = 0.8 * att[h]
    attblk = np.zeros((HC, H), np.float32)       # att_blk
    for h in range(H):
        attblk[h * C:(h + 1) * C, h] = att[h]
    wrab = np.concatenate([W_r, W_l @ attblk], axis=1)   # [F, 68]
    e30 = np.zeros((128, 128), np.float32)       # E30[r, a*4+h] = (r%32 == a)
    r = np.arange(128)
    for a in range(32):
        for h in range(H):
            e30[r[r % 32 == a], a * 4 + h] = 1.0

    blob = np.zeros((128, BLOB_COLS), np.float32)

    def put(nm, arr):
        w = arr.shape[1]
        blob[:arr.shape[0], _OFF[nm]:_OFF[nm] + w] = arr

    put("wl", W_l)
    put("wr", W_r)
    put("wrab", wrab)
    put("ident", np.eye(128, dtype=np.float32))
    put("biasb", np.broadcast_to(bias + b_r, (128, HC)))
    put("att2p8", att2p8)
    put("blp2", np.tile(b_l, 2).reshape(128, 1))
    put("brp2", np.tile(b_r, 2).reshape(128, 1))
    put("e30", e30)

    in_maps = []
    for core in range(NCORES):
        b, blk = core // 4, core % 4
        i0 = blk * NI
        cblob = blob.copy()
        # xin[p, k*128+f] = x[b, k*128+p, f]
        cblob[:, _OFF["xb"]:_OFF["xb"] + 8 * F] = (
            x[b].reshape(8, 128, F).transpose(1, 0, 2).reshape(128, 8 * F))
        cblob[:, _OFF["xis"]:_OFF["xis"] + 2 * F] = (
            x[b, i0:i0 + NI].reshape(2, 128, F).transpose(1, 0, 2).reshape(128, 2 * F))
        adjsl = adj[b, i0:i0 + NI, :].copy()
        adjsl[np.arange(NI), i0 + np.arange(NI)] = 1.0   # self loops
        adjm30 = (MASKVAL * (adjsl - 1.0)).astype(np.float16)
        # adjm[p, ib*N + j] = adjm30[ib*128 + p, j]
        adjm = adjm30.reshape(2, 128, N).transpose(1, 0, 2).reshape(128, 2 * N).copy()
        in_maps.append({"blob": cblob, "adjs": adjm})

    nc = _get_program()
    res = run_bass_kernel_spmd(nc, in_maps, core_ids=list(range(NCORES)))
    LAST_RESULTS = res
    outp = np.zeros((B, N, HC), np.float32)
    for core in range(NCORES):
        b, blk = core // 4, core % 4
        outp[b, blk * NI:(blk + 1) * NI, :] = res.results[core]["out"]
    return outp


# revision 12
# speedup vs baseline: 2.2624x; 1.0431x over previous
# DenseGATv2Conv Trainium2 kernel.
#
# Math (per batch b):
#   xl = x @ W_l + b_l ; xr = x @ W_r + b_r            [N, H*C]
#   alpha[i,j,h] = sum_c att[h,c] * leaky_relu(xl[j,hc] + xr[i,hc], 0.2)
#   S = softmax_j(alpha masked by adj(+self loops))
#   out[i,hc] = sum_j S[i,j,h] * xr[j,hc] + bias
#
# Key identity used on device:
#   leaky_relu(z) = 0.2*z + 0.8*relu(z)
#   alpha[i,j,h] = 0.2*sl[j,h] + 0.2*sr[i,h] + 0.8*sum_c att[h,c]*relu(xl[j,hc]+xr[i,hc])
# where sl = xl @ att_blk, sr = xr @ att_blk are rank-1 in the (i,j) plane.
# In softmax over j the exp(0.2*sr[i,h]) factor cancels; exp(0.2*sl[j,h]) is
# folded multiplicatively into the aggregation operand.
#
# Device scheme (v2): for each pair of destination rows (2 per "pair", 16
# pairs = 32 dest rows per "super"), DVE/Act/Pool build
#   rp[(d,hc), j] = relu(xl[j,hc] + xr[i_d,hc])         [128, N] f16
# Then the score contraction runs with rp as the matmul STATIONARY operand
# and a tiny [128, 8] att operand moving, so the result lands in PSUM
# already transposed: pst[j, (pair,d,h)].  The adjacency mask is folded in
# as an extra accumulating matmul that adds -30 to masked entries
# (adjm30 = 30*(adj-1) in {0,-30} f16, stationary; selection matrix E30
# moving), so a single exp() per super yields masked scores directly
# (exp(a-30) underflows f16 to 0).  The softmax denominator is computed in
# the aggregation matmul via an extra ones-like column (esl factor).
#
# Sharding: 8 cores = (batch b in 0..1) x (4 blocks of 256 destination rows).

import numpy as np

B, N, F, H, C = 2, 1024, 128, 4, 16
HC = H * C
NCORES = 8
NI = 256          # destination rows per core
NPAIR = NI // 2   # 128 pairs of destination rows
NSUP = 8          # supers of 16 pairs (32 dest rows) each
NEG = 0.2
MASKVAL = 30.0

_CACHE = {}
LAST_RESULTS = None

# engine assignment for the 16 rp builds of each super:
# 'd' = DVE tensor_scalar, 'a' = Act activation, 'p' = Pool tensor_scalar
RP_SCHED = "ddddddddddddaapp"

# blob column layout (f32 [128, BLOB_COLS]):
#   xb      [128, 8*128]   x[b] nodes, node k*128+p at cols k*128..k*128+128? no:
#                          xin[p, k*128+f] = x[k*128+p, f]
#   xis     [128, 2*128]   dest-row slice, same layout
#   wl      [128, 64]
#   wr      [128, 64]
#   ident   [128, 128]
#   biasb   [128, 64]
#   att2p8  [128, 8]
#   blp2    [128, 1]       b_l tiled twice
#   brp2    [128, 1]       b_r tiled twice
#   attb    [128, 4]       att_blk (rows 64:128 zero)
#   e30     [128, 128]     selection matrix for mask matmul
_OFF = {}
_c = 0
for _nm, _w in [("xis", 2 * F), ("wl", HC), ("wr", HC), ("wrab", HC + H),
                ("ident", 128), ("biasb", HC), ("att2p8", 8), ("blp2", 1),
                ("brp2", 1), ("e30", 128), ("xb", 8 * F)]:
    _OFF[_nm] = _c
    _c += _w
HEAD_COLS = _OFF["xb"]
BLOB_COLS = _c


def _build_program(debug=False):
    import concourse.bass as bass
    import concourse.mybir as mybir
    import concourse.tile as tile
    from concourse import bacc

    f32 = mybir.dt.float32
    f32r = mybir.dt.float32r
    f16 = mybir.dt.float16

    nc = bacc.Bacc(
        "TRN2",
        target_bir_lowering=False,
        debug=False,
        enable_asserts=False,
        num_devices=NCORES,
    )

    # ---- DRAM I/O ----
    blob = nc.dram_tensor("blob", [128, BLOB_COLS], f32, kind="ExternalInput").ap()
    adjs = nc.dram_tensor("adjs", [128, 2 * N], f16, kind="ExternalInput").ap()
    out = nc.dram_tensor("out", [NI, HC], f32, kind="ExternalOutput").ap()

    with tile.TileContext(nc) as tc:
        _body(tc, nc, mybir, bass, f32, f32r, f16, blob, adjs, out)

    nc.compile()
    return nc


def _body(tc, nc, mybir, bass, f32, f32r, f16, blob, adjs, out):
    from contextlib import ExitStack
    Alu = mybir.AluOpType
    Act = mybir.ActivationFunctionType
    ctx = ExitStack()
    with ctx:
        consts = ctx.enter_context(tc.tile_pool(name="consts", bufs=1))
        work = ctx.enter_context(tc.tile_pool(name="work", bufs=1))
        rp_pool = ctx.enter_context(tc.tile_pool(name="rp", bufs=2))
        outp = ctx.enter_context(tc.tile_pool(name="outp", bufs=2))
        psg = ctx.enter_context(tc.tile_pool(name="psg", bufs=1, space="PSUM"))
        pss = ctx.enter_context(tc.tile_pool(name="pss", bufs=2, space="PSUM"))
        pst = ctx.enter_context(tc.tile_pool(name="pst", bufs=2, space="PSUM"))

        dma = nc.sync.dma_start

        # ---------- load inputs ----------
        blob_t = consts.tile([128, BLOB_COLS], f32, tag="blob")
        dma(blob_t[:, 0:HEAD_COLS], blob[:, 0:HEAD_COLS])
        dma(blob_t[:, HEAD_COLS:], blob[:, HEAD_COLS:])
        adjm = consts.tile([128, 2 * N], f16, tag="adjm")   # 30*(adj-1), [i128][ib*N+j]
        dma(adjm[:], adjs)

        def bv(nm, w):
            return blob_t[:, _OFF[nm]:_OFF[nm] + w]

        xin = bv("xb", 8 * F)
        xis_t = bv("xis", 2 * F)
        id_t = bv("ident", 128)
        biasb_t = bv("biasb", HC)
        blp_t = blob_t[0:HC, _OFF["blp2"]:_OFF["blp2"] + 1]
        brp_t = blob_t[0:HC, _OFF["brp2"]:_OFF["brp2"] + 1]

        # f32r / f16 copies of small constants
        wl_r = consts.tile([F, HC], f32r, tag="wlr")
        wr_r = consts.tile([F, HC], f32r, tag="wrr")
        wrab_r = consts.tile([F, HC + H], f32r, tag="wrabr")
        att8_r = consts.tile([F, 8], f16, tag="att8r")
        e30_16 = consts.tile([128, 128], f16, tag="e30")
        nc.vector.tensor_copy(wl_r[:], bv("wl", HC))
        nc.vector.tensor_copy(wr_r[:], bv("wr", HC))
        nc.gpsimd.tensor_copy(wrab_r[:], bv("wrab", HC + H))
        nc.vector.tensor_copy(att8_r[:], bv("att2p8", 8))
        nc.gpsimd.tensor_copy(e30_16[:], bv("e30", 128))

        # ---------- xisT / xrsT / xrp (critical path to rp) ----------
        xisT = consts.tile([F, NI], f32r, tag="xisT")
        for k in range(2):
            pt = pst.tile([128, 128], f32, tag="pt")
            nc.tensor.transpose(pt[:], xis_t[:, k * F:(k + 1) * F], id_t)
            nc.vector.tensor_copy(xisT[:, k * 128:(k + 1) * 128], pt[:])
        xrsT = consts.tile([HC, NI], f32, tag="xrsT")
        g3 = psg.tile([128, N], f32, tag="g")
        pj3 = g3[0:HC, 0:NI]
        nc.tensor.matmul(pj3, wr_r[:], xisT[:], start=True, stop=True)
        nc.scalar.activation(xrsT[:], pj3, Act.Identity,
                             bias=brp_t, scale=1.0)
        xrp = consts.tile([128, NPAIR], f32, tag="xrp")
        ev = xrsT[:].rearrange("p (a two) -> p a two", two=2)
        nc.vector.tensor_copy(xrp[0:HC, :], ev[:, :, 0])
        nc.vector.tensor_copy(xrp[HC:128, :], ev[:, :, 1])

        # ---------- xT + xl2T ----------
        xT = consts.tile([F, N], f32r, tag="xT")              # [f, node]
        for k in range(8):
            pt = pst.tile([128, 128], f32, tag="pt")
            nc.tensor.transpose(pt[:], xin[:, k * F:(k + 1) * F], id_t)
            nc.vector.tensor_copy(xT[:, k * 128:(k + 1) * 128], pt[:])
        xl2T = consts.tile([128, N], f16, tag="xl2T")
        gp = psg.tile([128, N], f32, tag="g")
        pj = gp[0:HC, :]
        for half in range(2):
            s = slice(half * 512, (half + 1) * 512)
            nc.tensor.matmul(pj[:, s], wl_r[:], xT[:, s], start=True, stop=True)
        nc.scalar.activation(xl2T[0:HC, :], pj, Act.Identity,
                             bias=blp_t, scale=1.0)
        nc.vector.tensor_copy(xl2T[HC:128, :], xl2T[0:HC, :])

        # ---------- xr_mod: aggregation moving operand (natural layout) ----------
        # xr_mod[j, k*68 + h*17 + c] = xr_nob[j,hc]*esl[j,h] (c<16); c=16: esl[j,h]
        # xr_nob excludes b_r (folded into the output bias since sum_j Sbar = 1);
        # esl = exp(0.2 * x @ (W_l @ att_blk)) (the b_l@att_blk factor cancels).
        xr_mod = consts.tile([128, 8 * 68], f16, tag="xrmod")
        xrmv = xr_mod[:].rearrange("p (k h c) -> p k h c", k=8, h=H)

        def build_xr_mod_k(k):
            pkt = pst.tile([128, 128], f32, tag="pt", name="pk")
            pk = pkt[:, 0:HC + H]
            nc.tensor.matmul(pk, xT[:, k * 128:(k + 1) * 128], wrab_r[:],
                             start=True, stop=True)
            esl4 = work.tile([128, 8 * H], f16, tag="esl4", name="esl4")
            xr16 = work.tile([128, 8 * HC], f16, tag="xr16", name="xr16")
            nc.scalar.activation(esl4[:, k * H:(k + 1) * H],
                                 pkt[:, HC:HC + H], Act.Exp, scale=NEG)
            nc.scalar.copy(xr16[:, k * HC:(k + 1) * HC], pkt[:, 0:HC])
            nc.gpsimd.tensor_copy(xrmv[:, k, :, 16], esl4[:, k * H:(k + 1) * H])
            rep = esl4[:, k * H:(k + 1) * H].rearrange(
                "p (h one) -> p h one", one=1).broadcast_to([128, H, 16])
            srcx = xr16[:, k * HC:(k + 1) * HC].rearrange("p (h c) -> p h c", h=H)
            nc.gpsimd.tensor_tensor(xrmv[:, k, :, 0:16], srcx, rep, Alu.mult)

        # ---------- main streaming loop ----------
        # st_all[j, k*1024 + s*128 + a*4 + h], a = local dest (2*p+d) in super s
        # (k-major so the aggregation lhsT slice has ONE contiguous free dim)
        st_all = consts.tile([128, NSUP * N], f16, tag="stall")
        stv = st_all[:].rearrange("p (k s a h) -> p k s a h", k=8, s=NSUP, h=H)

        def super_iter(sup):
            ib, s4 = sup // 4, sup % 4
            if 1 <= sup <= 4:
                build_xr_mod_k(2 * (sup - 1))
                build_xr_mod_k(2 * (sup - 1) + 1)
            rp = rp_pool.tile([128, 16, 1024], f16, tag="rp")
            for p in range(16):
                gp = sup * 16 + p
                kind = RP_SCHED[p]
                if kind == "a":
                    nc.scalar.activation(rp[:, p, :], xl2T[:], Act.Relu,
                                         bias=xrp[:, gp:gp + 1], scale=1.0)
                elif kind == "p":
                    nc.gpsimd.tensor_scalar(rp[:, p, :], xl2T[:],
                                            xrp[:, gp:gp + 1],
                                            0.0, Alu.add, Alu.max)
                else:
                    nc.vector.tensor_scalar(rp[:, p, :], xl2T[:],
                                            xrp[:, gp:gp + 1],
                                            0.0, Alu.add, Alu.max)
            ps = pss.tile([128, 8, 128], f32, tag="sc")
            for k in range(8):
                # mask matmul: adds 30*(adj-1) (0 or -30) to every (j, a, h)
                nc.tensor.matmul(ps[:, k, :],
                                 adjm[s4 * 32:(s4 + 1) * 32,
                                      ib * N + k * 128: ib * N + (k + 1) * 128],
                                 e30_16[s4 * 32:(s4 + 1) * 32, :],
                                 start=True, stop=False, skip_group_check=True,
                                 tile_position=(s4 * 32, 0))
                for p in range(16):
                    nc.tensor.matmul(ps[:, k, p * 8:(p + 1) * 8],
                                     rp[:, p, k * 128:(k + 1) * 128],
                                     att8_r[:],
                                     start=False, stop=(p == 15),
                                     skip_group_check=True)
            nc.scalar.activation(stv[:, :, sup, :, :],
                                 ps[:].rearrange("p k (a h) -> p k a h", h=H), Act.Exp)

        # ---------- aggregation ----------
        # agg psum: one [128, 1024] tile per ib; head h uses cols h*32..h*32+17
        agg_ga = {}

        def agg_mms(ib, s0, ns, first):
            # accumulate supers [ib*4+s0, ib*4+s0+ns) -> dest rows s0*32..(s0+ns)*32
            if first:
                agg_ga[ib] = psg.tile([128, N], f32, tag="g", name="agg")
            ga = agg_ga[ib]
            for h in range(H):
                o = ga[s0 * 32:(s0 + ns) * 32, h * 32:h * 32 + 17]
                for k in range(8):
                    lhs = stv[:, k, ib * 4 + s0:ib * 4 + s0 + ns, :, h]
                    rhs = xr_mod[:, k * 68 + h * 17: k * 68 + (h + 1) * 17]
                    nc.tensor.matmul(o, lhs, rhs,
                                     start=(k == 0), stop=(k == 7),
                                     skip_group_check=True,
                                     tile_position=(0, s0 * 32))

        def agg_div(ib):
            ga = agg_ga[ib]
            out_f = outp.tile([128, HC], f32, tag="outf", name="outf")
            out_f2 = outp.tile([128, HC], f32, tag="outf2", name="outf2")
            for h in range(H):
                rz = work.tile([128, 1], f32, tag="rz", name="rz")
                nc.vector.reciprocal(rz[:], ga[:, h * 32 + 16:h * 32 + 17])
                nc.vector.tensor_scalar(out_f[:, h * 16:(h + 1) * 16],
                                        ga[:, h * 32:h * 32 + 16],
                                        rz[:, 0:1], None, Alu.mult)
            nc.vector.tensor_add(out_f2[:], out_f[:], biasb_t)
            dma(out[ib * 128:(ib + 1) * 128, :], out_f2[:])

        for sup in range(NSUP):
            super_iter(sup)
            if sup == 5:
                agg_mms(0, 0, 4, True)
            elif sup == 6:
                agg_div(0)
                agg_mms(1, 0, 3, True)
        agg_mms(1, 3, 1, False)
        agg_div(1)


def _get_program():
    if "nc" not in _CACHE:
        _CACHE["nc"] = _build_program()
    return _CACHE["nc"]


def kernel(x, adj, W_l, b_l, W_r, b_r, att, bias):
    global LAST_RESULTS
    from concourse.bass_utils import run_bass_kernel_spmd

    x = np.ascontiguousarray(np.asarray(x, dtype=np.float32))
    adj = np.ascontiguousarray(np.asarray(adj, dtype=np.float32))
    W_l = np.asarray(W_l, dtype=np.float32)
    b_l = np.asarray(b_l, dtype=np.float32)
    W_r = np.asarray(W_r, dtype=np.float32)
    b_r = np.asarray(b_r, dtype=np.float32)
    att = np.asarray(att, dtype=np.float32)
    bias = np.asarray(bias, dtype=np.float32)

    # host-side constant prep
    att2p8 = np.zeros((F, 8), np.float32)        # 0.8 * att, block diagonal x2
    for d in range(2):
        for h in range(H):
            att2p8[d * HC + h * C:(d * HC + (h + 1) * C), d * H + h] = 0.8 * att[h]
    attblk = np.zeros((HC, H), np.float32)       # att_blk
    for h in range(H):
        attblk[h * C:(h + 1) * C, h] = att[h]
    wrab = np.concatenate([W_r, W_l @ attblk], axis=1)   # [F, 68]
    e30 = np.zeros((128, 128), np.float32)       # E30[r, a*4+h] = (r%32 == a)
    r = np.arange(128)
    for a in range(32):
        for h in range(H):
            e30[r[r % 32 == a], a * 4 + h] = 1.0

    blob = np.zeros((128, BLOB_COLS), np.float32)

    def put(nm, arr):
        w = arr.shape[1]
        blob[:arr.shape[0], _OFF[nm]:_OFF[nm] + w] = arr

    put("wl", W_l)
    put("wr", W_r)
    put("wrab", wrab)
    put("ident", np.eye(128, dtype=np.float32))
    put("biasb", np.broadcast_to(bias + b_r, (128, HC)))
    put("att2p8", att2p8)
    put("blp2", np.tile(b_l, 2).reshape(128, 1))
    put("brp2", np.tile(b_r, 2).reshape(128, 1))
    put("e30", e30)

    in_maps = []
    for core in range(NCORES):
        b, blk = core // 4, core % 4
        i0 = blk * NI
        cblob = blob.copy()
        # xin[p, k*128+f] = x[b, k*128+p, f]
        cblob[:, _OFF["xb"]:_OFF["xb"] + 8 * F] = (
            x[b].reshape(8, 128, F).transpose(1, 0, 2).reshape(128, 8 * F))
        cblob[:, _OFF["xis"]:_OFF["xis"] + 2 * F] = (
            x[b, i0:i0 + NI].reshape(2, 128, F).transpose(1, 0, 2).reshape(128, 2 * F))
        adjsl = adj[b, i0:i0 + NI, :].copy()
        adjsl[np.arange(NI), i0 + np.arange(NI)] = 1.0   # self loops
        adjm30 = (MASKVAL * (adjsl - 1.0)).astype(np.float16)
        # adjm[p, ib*N + j] = adjm30[ib*128 + p, j]
        adjm = adjm30.reshape(2, 128, N).transpose(1, 0, 2).reshape(128, 2 * N).copy()
        in_maps.append({"blob": cblob, "adjs": adjm})

    nc = _get_program()
    res = run_bass_kernel_spmd(nc, in_maps, core_ids=list(range(NCORES)))
    LAST_RESULTS = res
    outp = np.zeros((B, N, HC), np.float32)
    for core in range(NCORES):
        b, blk = core // 4, core % 4
        outp[b, blk * NI:(blk + 1) * NI, :] = res.results[core]["out"]
    return outp
